# revision 1
# baseline (speedup 1.0000x reference)
"""Trainium2 Bass kernel for the pairwise-MLP geometric convolution.

Reference computes, per batch z:
    rel[a,b]   = g[b] - g[a]
    h[a,b,:]   = relu(rel @ W1 + b1)                      [N,N,H]
    k[a,b,:]   = h @ W2 + b2  -> [N,N,C_OUT,C_IN]
    out[a,i]   = sum_{b,j} k[a,b,i,j] * f[b,j]

Key factorization (avoids materializing k, 537MB -> ~1MB):
    U = g @ W1  (so rel@W1 = U[b]-U[a])
    G[b,h,i]   = sum_j W2[h, i*C_IN+j] * f[b,j]
    out[a,i]   = sum_{b,h} relu(U[b,h]+b1[h]-U[a,h]) * G[b,h,i]
               + sum_j b2[i,j] * (sum_b f[b,j])

Sharding over 8 cores: z (2) x b-quarter (4). Each core computes the full
[i=32, a=256] transposed partial for its 64 b's; host sums quarters and
transposes. Contraction runs on the PE as 32 accumulating matmuls with
K=128 chunks of (b-pair x 64 h): stationary G chunk [128,32], moving
T chunk [128,256] built by one fused tensor_scalar (add bias, relu) per
chunk, spread across DVE/ACT/GPSIMD.

Hardware constraint honored throughout: a PE Matmult can carry at most ONE
sync-wait, so all small inputs arrive in a single packed DMA, and two
dummy matmuls make the PE observe the two g_sb gather DMAs before the
main accumulation chain.
"""

import os
import sys

import numpy as np

_TRN_REPO = "/opt/trn_rl_repo"
if _TRN_REPO not in sys.path:
    sys.path.insert(0, _TRN_REPO)

from contextlib import ExitStack

import concourse.bass as bass
import concourse.mybir as mybir
import concourse.tile as tile
from concourse.bass_utils import run_bass_kernel_spmd

from concourse.vector_clock import ScopedClock

# The walrus codegen used on the axon/PJRT path accepts at most ONE sync-wait
# per TPB instruction. Tile's kernel-tail drain aggregates a wait for every
# live semaphore onto a single Drain, which walrus rejects. Patch the tail to
# spread those waits across single-wait SP nops before an unadorned drain.
_orig_drain_and_barrier = tile.TileContext._drain_and_barrier


def _split_wait_drain_and_barrier(self, tick_clock, wait_clock):
    nc = self.nc
    probe = nc.sync.nop(nofuse=True)
    wait_clock.add_sem_waits(probe.ins, ScopedClock({None: tick_clock.global_clock}))
    si = probe.ins.sync_info
    waits = list(si.on_wait) if si is not None and si.on_wait else []
    if len(waits) > 1:
        probe.ins.sync_info = mybir.SyncInfo(on_wait=waits[:1], on_update=[])
        for w in waits[1:]:
            extra = nc.sync.nop(nofuse=True)
            extra.ins.sync_info = mybir.SyncInfo(on_wait=[w], on_update=[])
    nc.sync.drain()
    nc.all_engine_barrier()
    popped = nc._tile_sem_poison_stack.pop()
    assert popped is self._sem_poison
    nc.clear_and_free_semaphores(list(self.sems.allocated().values()))
    nc.all_engine_barrier()


tile.TileContext._drain_and_barrier = _split_wait_drain_and_barrier

F32 = mybir.dt.float32
# bf16 runs the PE at 1 cycle/row vs 4 for fp32; accumulation stays fp32 in
# PSUM. Only the big contraction operands (T, G) are bf16.
BF16 = mybir.dt.bfloat16
Z, N, C_IN, C_OUT, H = 2, 256, 32, 32, 64
BQ = 64          # b-points per core (N / 4 quarters)
NPAIR = BQ // 2  # 32 K-chunks of (2 b x 64 h) = 128

# packed small-input tensor layout (fp32): [64, PKW]
#   cols 0:256    fTfull   (parts 0:32)
#   cols 256:288  b2T      (parts 0:32)
#   col  288      b1c      (parts 0:64)
PKW = 289
# bf16 packed tensor (matmul operands), loaded as two DMAs (cols 0:MA,
# MA:MPW) so the U and first G' matmuls start before the whole tensor lands:
#   cols 0:64       fTq      (parts 0:32)
#   cols 64:320     gT       (parts 0:3)
#   cols 320:384    gTb      (parts 0:3)
#   cols 384:448    W1       (parts 0:3)
#   cols 448:2496   M2p
MPW = 2496
MA = 1472

# engine for each of the 32 T-chunk builds: v=vector(DVE), s=scalar(ACT),
# g=gpsimd. ACT carries the shared prep, DVE the G copies.
T_ENGINES = ["g", "s", "v", "g", "s", "g", "s", "v"] * 4
# PE warm-up matmuls between the G' matmuls and the main chain.
N_WARMERS = 16


def build_nc(debug: bool = False) -> bass.Bass:
    nc = bass.Bass("TRN2", target_bir_lowering=False, debug=debug, num_devices=8)

    m2p = nc.dram_tensor("M2p", [C_IN, MPW], BF16, kind="ExternalInput").ap()
    pk = nc.dram_tensor("pk", [H, PKW], F32, kind="ExternalInput").ap()
    outp = nc.dram_tensor("outp", [C_OUT, N], F32, kind="ExternalOutput").ap()

    with tile.TileContext(nc) as tc, ExitStack() as ctx:
        consts = ctx.enter_context(tc.tile_pool(name="consts", bufs=1))
        work = ctx.enter_context(tc.tile_pool(name="work", bufs=1))
        # bufs=NPAIR: every T tile gets its own slot, so no T-op ever waits
        # for a PE slot release (keeps every instruction at <=1 sync wait,
        # a walrus codegen hard limit).
        tpool = ctx.enter_context(tc.tile_pool(name="tpool", bufs=NPAIR))
        psum = ctx.enter_context(tc.tile_pool(name="psum", bufs=1, space="PSUM"))
        dpool = ctx.enter_context(tc.tile_pool(name="dpool", bufs=1, space="DRAM"))

        # ---- input loads. pk goes through the Pool SWDGE queue so the SP
        # HWDGE ring stays within 8 DMAs (no semaphore-lane reuse).
        m2p_sb = consts.tile([C_IN, MPW], BF16)
        nc.sync.dma_start(out=m2p_sb[:, 0:MA], in_=m2p[:, 0:MA])
        nc.sync.dma_start(out=m2p_sb[:, MA:MPW], in_=m2p[:, MA:MPW])
        pk_sb = consts.tile([H, PKW], F32)
        nc.gpsimd.dma_start(out=pk_sb, in_=pk)

        fTq_bf = m2p_sb[:, 0:64]
        gT_bf = m2p_sb[0:3, 64:320]
        gTb_bf = m2p_sb[0:3, 320:384]
        w1_bf = m2p_sb[0:3, 384:448]
        fTfull_sb = pk_sb[0:C_IN, 0:256]
        b2t_sb = pk_sb[0:C_IN, 256:288]
        b1_sb = pk_sb[0:H, 288:289]

        # First DVE / ACT ops must observe only the pk DMA semaphore.
        scol = work.tile([C_IN, 1], F32)
        nc.vector.tensor_reduce(out=scol, in_=fTfull_sb,
                                axis=mybir.AxisListType.X, op=mybir.AluOpType.add)
        s_bcast = work.tile([C_IN, N], BF16)
        nc.vector.tensor_scalar(out=s_bcast, in0=scol.broadcast_to([C_IN, N]),
                                scalar1=0.0, scalar2=None,
                                op0=mybir.AluOpType.add)
        b2t_bf = work.tile([C_IN, C_OUT], BF16)
        nc.vector.tensor_copy(b2t_bf, b2t_sb)

        # ---- U matmuls: U^T = W1^T @ g^T (bf16 in, fp32 accumulate).
        # Both U results share one PSUM bank, freeing a bank for the
        # warm-up matmuls.
        u_ps = psum.tile([H, N + BQ], F32)
        uaT_ps = u_ps[:, 0:N]
        ubT_ps = u_ps[:, N:N + BQ]
        nc.tensor.matmul(uaT_ps, lhsT=w1_bf, rhs=gT_bf, start=True, stop=True)
        nc.tensor.matmul(ubT_ps, lhsT=w1_bf, rhs=gTb_bf, start=True, stop=True)

        # All shared T-op inputs are produced on ACT so T consumers on any
        # engine need exactly one (ACT) wait. negUa duplicated on both
        # partition halves: [128, N].
        negua2 = work.tile([2 * H, N], F32)
        nc.scalar.activation(negua2[0:H, :], uaT_ps,
                             mybir.ActivationFunctionType.Copy, scale=-1.0)
        nc.scalar.activation(negua2[H:2 * H, :], uaT_ps,
                             mybir.ActivationFunctionType.Copy, scale=-1.0)

        # Ub + b1, then stacked by pair: ubT2[bl*H+h, p] = Ub[2p+bl, h] + b1[h]
        ubB = work.tile([H, BQ], F32)
        nc.vector.tensor_scalar(out=ubB, in0=ubT_ps, scalar1=b1_sb,
                                scalar2=None, op0=mybir.AluOpType.add)
        ubT2 = work.tile([2 * H, NPAIR], F32)
        ubB_r = ubB.rearrange("h (p two) -> h two p", two=2)
        nc.scalar.activation(ubT2[0:H, :], ubB_r[:, 0, :],
                             mybir.ActivationFunctionType.Copy)
        nc.scalar.activation(ubT2[H:2 * H, :], ubB_r[:, 1, :],
                             mybir.ActivationFunctionType.Copy)

        # ---- G: G'[b, h*32+i] = sum_j fTq[j,b] * M2p[j, h*32+i] ----
        g_ps = []
        for k in range(4):
            gp = psum.tile([BQ, 512], F32, name=f"g_ps{k}", tag=f"g_ps{k}")
            nc.tensor.matmul(gp, lhsT=fTq_bf,
                             rhs=m2p_sb[:, 448 + k * 512:448 + (k + 1) * 512],
                             start=True, stop=True)
            g_ps.append(gp)

        # PSUM -> SBUF on DVE (DMA cannot read PSUM), then bounce through
        # DRAM to regroup (b-pair, h) onto partitions.
        g_tmp = work.tile([BQ, H * C_OUT], BF16)
        for k in range(4):
            nc.vector.tensor_copy(g_tmp[:, k * 512:(k + 1) * 512], g_ps[k])
        g_sb = work.tile([2 * H, NPAIR, C_OUT], BF16)
        g_dram = dpool.tile([BQ, H * C_OUT], BF16)
        nc.sync.dma_start(out=g_dram, in_=g_tmp)
        # Two gathers split by p-half. Because 64 h * 32 i = 2048 = the
        # g_dram row stride, the (bl, h) pair merges into ONE uniform
        # stride-32 dim, keeping each side a legal 3D AP:
        #   src element (2p+bl, h*32+i) -> offset (bl*64+h)*32 + p*4096 + i
        g0 = g_dram[:, :]
        for ph in range(2):
            g_src = bass.AP(tensor=g0.tensor,
                            offset=g0.offset + ph * 16 * 4096,
                            ap=[[32, 2 * H], [4096, 16], [1, C_OUT]])
            nc.sync.dma_start(out=g_sb[:, 16 * ph:16 * (ph + 1), :],
                              in_=g_src)

        # ---- b2 bias term first in the acc group ----
        acc = psum.tile([C_OUT, N], F32)
        nc.tensor.matmul(acc, lhsT=b2t_bf, rhs=s_bcast, start=True, stop=False)

        scrap = psum.tile([C_OUT, 1], F32)

        def observe_gather(ph):
            # PE observes the p-half gather (one wait) so the following
            # main matmuls need only their T-tile wait.
            nc.tensor.matmul(scrap, lhsT=g_sb[:, 16 * ph, :],
                             rhs=g_sb[:, 16 * ph, 0:1],
                             start=True, stop=True)

        # ---- main contraction: acc[i, a] += G_p^T @ T_p ----
        # T-gated PE warm-up: warmer w consumes t_w as it is produced, so
        # the PE tracks T production (staying at high p-state) instead of
        # idling while the G gathers are in flight.
        warm_ps = psum.tile([C_OUT, N], F32)
        t_tiles = []
        for p in range(NPAIR):
            t_p = tpool.tile([2 * H, N], BF16, tag="T", name=f"t_{p}")
            t_tiles.append(t_p)
            eng = T_ENGINES[p]
            if eng == "s":
                nc.scalar.activation(t_p, negua2,
                                     mybir.ActivationFunctionType.Relu,
                                     bias=ubT2[:, p:p + 1], scale=1.0)
            else:
                e = nc.vector if eng == "v" else nc.gpsimd
                e.tensor_scalar(out=t_p, in0=negua2,
                                scalar1=ubT2[:, p:p + 1], scalar2=0.0,
                                op0=mybir.AluOpType.add,
                                op1=mybir.AluOpType.max)
            if p < N_WARMERS:
                nc.tensor.matmul(warm_ps, lhsT=t_p[0:C_IN, 0:C_OUT],
                                 rhs=t_p[0:C_IN, :], start=True, stop=True)
        for ph in range(2):
            observe_gather(ph)
            for p in range(16 * ph, 16 * (ph + 1)):
                nc.tensor.matmul(acc, lhsT=g_sb[:, p, :], rhs=t_tiles[p],
                                 start=False, stop=(p == NPAIR - 1))

        # ---- store ----
        out_sb = work.tile([C_OUT, N], F32)
        nc.scalar.activation(out_sb, acc, mybir.ActivationFunctionType.Copy)
        nc.sync.dma_start(out=outp, in_=out_sb)

    return nc


def shard_inputs(features, geometry, W1, b1, W2, b2) -> list[dict]:
    import ml_dtypes
    bf16 = ml_dtypes.bfloat16
    f = np.ascontiguousarray(np.asarray(features, np.float32))
    g = np.ascontiguousarray(np.asarray(geometry, np.float32))
    W1 = np.ascontiguousarray(np.asarray(W1, np.float32))
    b1 = np.ascontiguousarray(np.asarray(b1, np.float32))
    W2 = np.ascontiguousarray(np.asarray(W2, np.float32))
    b2 = np.ascontiguousarray(np.asarray(b2, np.float32))

    m2p = W2.reshape(H, C_OUT, C_IN).transpose(2, 0, 1).reshape(C_IN, H * C_OUT)
    b2t = np.ascontiguousarray(b2.reshape(C_OUT, C_IN).T)

    maps = []
    for core in range(8):
        z, q = divmod(core, 4)
        sl = slice(q * BQ, (q + 1) * BQ)
        pk = np.zeros((H, PKW), np.float32)
        pk[0:C_IN, 0:256] = f[z].T
        if q == 0:
            pk[0:C_IN, 256:288] = b2t
        pk[0:H, 288] = b1
        mp = np.zeros((C_IN, MPW), bf16)
        mp[:, 0:64] = f[z, sl].T.astype(bf16)
        mp[0:3, 64:320] = g[z].T.astype(bf16)
        mp[0:3, 320:384] = g[z, sl].T.astype(bf16)
        mp[0:3, 384:448] = W1.astype(bf16)
        mp[:, 448:2496] = m2p.astype(bf16)
        maps.append({"pk": pk, "M2p": mp})
    return maps


def unshard(parts: list[np.ndarray]) -> np.ndarray:
    out = np.empty((Z, N, C_OUT), np.float32)
    for z in range(Z):
        acc = parts[4 * z].astype(np.float32)
        for q in range(1, 4):
            acc = acc + parts[4 * z + q]
        out[z] = acc.T
    return out


def kernel(**inputs) -> np.ndarray:
    nc = build_nc(debug=False)
    in_maps = shard_inputs(**inputs)
    res = run_bass_kernel_spmd(nc, in_maps, list(range(8)))
    return unshard([r["outp"] for r in res.results])



# revision 27
# speedup vs baseline: 1.1747x; 1.1747x over previous
"""Trainium2 Bass kernel for the pairwise-MLP geometric convolution.

Factorization (per core: one z, one b-quarter of 64 points):
    U = g @ W1;  G[b,h,i] = sum_j W2[h, i*C_IN+j] f[b,j]
    out[a,i] = sum_{b,h} relu(U[b,h]+b1[h]-U[a,h]) * G[b,h,i]
             + sum_j b2[i,j] * (sum_b f[b,j])

Structure follows the HW-proven DRAM-bounce regroup for G, with:
  - PSUM->SBUF copies of G\' split across DVE and ACT; the bounce DMA and
    the gathers issue from SP where sentinel nops absorb extra sync waits
    (SP SEQ is in-order, so a wait on an earlier nop still happens-before).
  - negua2 in bf16 so DVE T-builds hit the 4x perf mode; T engine split
    rebalanced DVE-heavy.
  - Main contraction in out[a,i] layout: per b-pair two accumulating
    matmuls with lhsT = T_p a-half (128 wide) and rhs = g_sb[:,p,:]
    (32 cols) -> 64 matmuls of ~13ns instead of 32 of ~107ns.
  - b2 term via lhsT = s_bcast half (materialized fsum broadcast).
"""

import sys

_TRN_REPO = "/opt/trn_rl_repo"
if _TRN_REPO not in sys.path:
    sys.path.insert(0, _TRN_REPO)

from contextlib import ExitStack

import numpy as np

import concourse.bass as bass
import concourse.mybir as mybir
import concourse.tile as tile
from concourse.bass_utils import run_bass_kernel_spmd

from concourse.vector_clock import ScopedClock

_orig_drain_and_barrier = tile.TileContext._drain_and_barrier


def _split_wait_drain_and_barrier(self, tick_clock, wait_clock):
    nc = self.nc
    probe = nc.sync.nop(nofuse=True)
    wait_clock.add_sem_waits(probe.ins, ScopedClock({None: tick_clock.global_clock}))
    si = probe.ins.sync_info
    waits = list(si.on_wait) if si is not None and si.on_wait else []
    if len(waits) > 1:
        probe.ins.sync_info = mybir.SyncInfo(on_wait=waits[:1], on_update=[])
        for w in waits[1:]:
            extra = nc.sync.nop(nofuse=True)
            extra.ins.sync_info = mybir.SyncInfo(on_wait=[w], on_update=[])
    nc.sync.drain()
    nc.all_engine_barrier()
    popped = nc._tile_sem_poison_stack.pop()
    assert popped is self._sem_poison
    nc.clear_and_free_semaphores(list(self.sems.allocated().values()))
    nc.all_engine_barrier()


tile.TileContext._drain_and_barrier = _split_wait_drain_and_barrier

F32 = mybir.dt.float32
BF16 = mybir.dt.bfloat16
Z, N, C_IN, C_OUT, H = 2, 256, 32, 32, 64
BQ = 64
NPAIR = BQ // 2

PKW = 289
MPW = 2496
MA = 1472

T_ENGINES = (
    ["g", "v", "v", "s", "v", "g", "v", "v",
     "s", "v", "g", "v", "v", "s", "v", "g",
     "v", "s", "v", "g", "v", "v", "s", "v",
     "g", "v", "s", "v", "g", "v", "v", "v"]
)
N_WARMERS = 16


def build_nc(debug: bool = False) -> bass.Bass:
    nc = bass.Bass("TRN2", target_bir_lowering=False, debug=debug, num_devices=8)

    m2p = nc.dram_tensor("M2p", [C_IN, MPW], BF16, kind="ExternalInput").ap()
    pk = nc.dram_tensor("pk", [H, PKW], F32, kind="ExternalInput").ap()
    outp = nc.dram_tensor("outp", [N, C_OUT], F32, kind="ExternalOutput").ap()

    with tile.TileContext(nc) as tc, ExitStack() as ctx:
        consts = ctx.enter_context(tc.tile_pool(name="consts", bufs=1))
        work = ctx.enter_context(tc.tile_pool(name="work", bufs=1))
        tpool = ctx.enter_context(tc.tile_pool(name="tpool", bufs=NPAIR))
        psum = ctx.enter_context(tc.tile_pool(name="psum", bufs=1, space="PSUM"))
        dpool = ctx.enter_context(tc.tile_pool(name="dpool", bufs=1, space="DRAM"))

        m2p_sb = consts.tile([C_IN, MPW], BF16)
        nc.sync.dma_start(out=m2p_sb[:, 0:MA], in_=m2p[:, 0:MA])
        nc.sync.dma_start(out=m2p_sb[:, MA:MPW], in_=m2p[:, MA:MPW])
        pk_sb = consts.tile([H, PKW], F32)
        nc.gpsimd.dma_start(out=pk_sb, in_=pk)

        fTq_bf = m2p_sb[:, 0:64]
        gT_bf = m2p_sb[0:3, 64:320]
        gTb_bf = m2p_sb[0:3, 320:384]
        w1_bf = m2p_sb[0:3, 384:448]
        fTfull_sb = pk_sb[0:C_IN, 0:256]
        b2t_sb = pk_sb[0:C_IN, 256:288]
        b1_sb = pk_sb[0:H, 288:289]

        scol = work.tile([C_IN, 1], F32)
        nc.vector.tensor_reduce(out=scol, in_=fTfull_sb,
                                axis=mybir.AxisListType.X, op=mybir.AluOpType.add)
        s_bcast = work.tile([C_IN, N], BF16)
        nc.vector.tensor_scalar(out=s_bcast, in0=scol.broadcast_to([C_IN, N]),
                                scalar1=0.0, scalar2=None,
                                op0=mybir.AluOpType.add)
        b2t_bf = work.tile([C_IN, C_OUT], BF16)
        nc.vector.tensor_copy(b2t_bf, b2t_sb)

        u_ps = psum.tile([H, N + BQ], F32)
        uaT_ps = u_ps[:, 0:N]
        ubT_ps = u_ps[:, N:N + BQ]
        nc.tensor.matmul(uaT_ps, lhsT=w1_bf, rhs=gT_bf, start=True, stop=True)
        nc.tensor.matmul(ubT_ps, lhsT=w1_bf, rhs=gTb_bf, start=True, stop=True)

        negua2 = work.tile([2 * H, N], BF16)
        nc.scalar.activation(negua2[0:H, :], uaT_ps,
                             mybir.ActivationFunctionType.Copy, scale=-1.0)
        nc.scalar.activation(negua2[H:2 * H, :], uaT_ps,
                             mybir.ActivationFunctionType.Copy, scale=-1.0)

        ubB = work.tile([H, BQ], F32)
        nc.vector.tensor_scalar(out=ubB, in0=ubT_ps, scalar1=b1_sb,
                                scalar2=None, op0=mybir.AluOpType.add)
        ubT2 = work.tile([2 * H, NPAIR], F32)
        ubB_r = ubB.rearrange("h (p two) -> h two p", two=2)
        nc.scalar.activation(ubT2[0:H, :], ubB_r[:, 0, :],
                             mybir.ActivationFunctionType.Copy)
        nc.scalar.activation(ubT2[H:2 * H, :], ubB_r[:, 1, :],
                             mybir.ActivationFunctionType.Copy)

        g_ps = []
        for k in range(4):
            gp = psum.tile([BQ, 512], F32, name=f"g_ps{k}", tag=f"g_ps{k}")
            nc.tensor.matmul(gp, lhsT=fTq_bf,
                             rhs=m2p_sb[:, 448 + k * 512:448 + (k + 1) * 512],
                             start=True, stop=True)
            g_ps.append(gp)

        g_tmp = work.tile([BQ, H * C_OUT], BF16)
        for k, eng in ((0, "v"), (1, "s"), (2, "s"), (3, "v")):
            dst = g_tmp[:, k * 512:(k + 1) * 512]
            if eng == "v":
                nc.vector.tensor_copy(dst, g_ps[k])
            else:
                nc.scalar.activation(dst, g_ps[k],
                                     mybir.ActivationFunctionType.Copy)

        g_sb = work.tile([2 * H, NPAIR, C_OUT], BF16)
        g_dram = dpool.tile([BQ, H * C_OUT], BF16)
        sponges = [nc.sync.nop(nofuse=True).ins for _ in range(4)]
        nc.sync.dma_start(out=g_dram, in_=g_tmp)
        g0 = g_dram[:, :]
        for ph in range(2):
            g_src = bass.AP(tensor=g0.tensor,
                            offset=g0.offset + ph * 16 * 4096,
                            ap=[[32, 2 * H], [4096, 16], [1, C_OUT]])
            nc.sync.dma_start(out=g_sb[:, 16 * ph:16 * (ph + 1), :],
                              in_=g_src)

        acc = psum.tile([128, 2 * C_OUT], F32)
        nc.tensor.matmul(acc[:, 0:C_OUT], lhsT=s_bcast[:, 0:128], rhs=b2t_bf,
                         start=True, stop=False)
        nc.tensor.matmul(acc[:, C_OUT:2 * C_OUT], lhsT=s_bcast[:, 128:256],
                         rhs=b2t_bf, start=True, stop=False)

        scrap = psum.tile([C_OUT, 1], F32)

        def observe_gather(ph):
            nc.tensor.matmul(scrap, lhsT=g_sb[:, 16 * ph, :],
                             rhs=g_sb[:, 16 * ph, 0:1],
                             start=True, stop=True)

        warm_ps = psum.tile([C_OUT, N], F32)
        t_tiles = []
        for p in range(NPAIR):
            t_p = tpool.tile([2 * H, N], BF16, tag="T", name=f"t_{p}")
            t_tiles.append(t_p)
            eng = T_ENGINES[p]
            if eng == "s":
                nc.scalar.activation(t_p, negua2,
                                     mybir.ActivationFunctionType.Relu,
                                     bias=ubT2[:, p:p + 1], scale=1.0)
            else:
                e = nc.vector if eng == "v" else nc.gpsimd
                e.tensor_scalar(out=t_p, in0=negua2,
                                scalar1=ubT2[:, p:p + 1], scalar2=0.0,
                                op0=mybir.AluOpType.add,
                                op1=mybir.AluOpType.max)
            if p < N_WARMERS:
                nc.tensor.matmul(warm_ps, lhsT=t_p[0:C_IN, 0:C_OUT],
                                 rhs=t_p[0:C_IN, :], start=True, stop=True)
        for ph in range(2):
            observe_gather(ph)
            for p in range(16 * ph, 16 * (ph + 1)):
                rhs = g_sb[:, p, :]
                last = p == NPAIR - 1
                nc.tensor.matmul(acc[:, 0:C_OUT], lhsT=t_tiles[p][:, 0:128],
                                 rhs=rhs, start=False, stop=last)
                nc.tensor.matmul(acc[:, C_OUT:2 * C_OUT],
                                 lhsT=t_tiles[p][:, 128:256],
                                 rhs=rhs, start=False, stop=last)

        out_sb = work.tile([128, 2 * C_OUT], F32)
        nc.vector.tensor_copy(out_sb[:, 0:C_OUT], acc[:, 0:C_OUT])
        nc.vector.tensor_copy(out_sb[:, C_OUT:2 * C_OUT],
                              acc[:, C_OUT:2 * C_OUT])
        sponges += [nc.sync.nop(nofuse=True).ins for _ in range(2)]
        dst = bass.AP(tensor=outp.tensor, offset=outp.offset,
                      ap=[[C_OUT, 128], [128 * C_OUT, 2], [1, C_OUT]])
        nc.sync.dma_start(out=dst,
                          in_=out_sb.rearrange("p (h i) -> p h i", i=C_OUT))

    sponge_names = {sp.name for sp in sponges}
    for blk in nc.m.functions[0].blocks:
        insts = list(blk.instructions)
        for idx, ins in enumerate(insts):
            si = ins.sync_info
            if si is None or not si.on_wait or len(si.on_wait) <= 1:
                continue
            waits = list(si.on_wait)
            assert str(ins.engine).endswith("SP"), (
                f"multi-wait on non-SP instruction {ins.name} "
                f"{ins.engine} {ins.opcode}: "
                f"{[(w.ant_name, w.wait_value) for w in waits]}"
            )
            extras, keep = waits[:-1], waits[-1:]
            j = idx - 1
            while extras and j >= 0:
                prev = insts[j]
                j -= 1
                if prev.name not in sponge_names:
                    continue
                psi = prev.sync_info
                if psi is not None and psi.on_wait:
                    continue
                prev.sync_info = mybir.SyncInfo(on_wait=[extras.pop()],
                                                on_update=[])
            assert not extras, f"no sponge for {ins.name}"
            ins.sync_info = mybir.SyncInfo(on_wait=keep,
                                           on_update=list(si.on_update or []))
    return nc


def shard_inputs(features, geometry, W1, b1, W2, b2) -> list[dict]:
    import ml_dtypes
    bf16 = ml_dtypes.bfloat16
    f = np.ascontiguousarray(np.asarray(features, np.float32))
    g = np.ascontiguousarray(np.asarray(geometry, np.float32))
    W1 = np.ascontiguousarray(np.asarray(W1, np.float32))
    b1 = np.ascontiguousarray(np.asarray(b1, np.float32))
    W2 = np.ascontiguousarray(np.asarray(W2, np.float32))
    b2 = np.ascontiguousarray(np.asarray(b2, np.float32))

    m2p = W2.reshape(H, C_OUT, C_IN).transpose(2, 0, 1).reshape(C_IN, H * C_OUT)
    b2t = np.ascontiguousarray(b2.reshape(C_OUT, C_IN).T)

    maps = []
    for core in range(8):
        z, q = divmod(core, 4)
        sl = slice(q * BQ, (q + 1) * BQ)
        pkv = np.zeros((H, PKW), np.float32)
        pkv[0:C_IN, 0:256] = f[z].T
        if q == 0:
            pkv[0:C_IN, 256:288] = b2t
        pkv[0:H, 288] = b1
        mp = np.zeros((C_IN, MPW), bf16)
        mp[:, 0:64] = f[z, sl].T.astype(bf16)
        mp[0:3, 64:320] = g[z].T.astype(bf16)
        mp[0:3, 320:384] = g[z, sl].T.astype(bf16)
        mp[0:3, 384:448] = W1.astype(bf16)
        mp[:, 448:2496] = m2p.astype(bf16)
        maps.append({"pk": pkv, "M2p": mp})
    return maps


def unshard(parts: list[np.ndarray]) -> np.ndarray:
    out = np.empty((Z, N, C_OUT), np.float32)
    for z in range(Z):
        acc = parts[4 * z].astype(np.float32)
        for q in range(1, 4):
            acc = acc + parts[4 * z + q]
        out[z] = acc
    return out


def kernel(**inputs) -> np.ndarray:
    nc = build_nc(debug=False)
    in_maps = shard_inputs(**inputs)
    res = run_bass_kernel_spmd(nc, in_maps, list(range(8)))
    return unshard([r["outp"] for r in res.results])


# revision 29
# speedup vs baseline: 1.1964x; 1.0185x over previous
"""Trainium2 Bass kernel for the pairwise-MLP geometric convolution.

Factorization (per core: one z, one b-quarter of 64 points):
    U = g @ W1;  G[b,h,i] = sum_j W2[h, i*C_IN+j] f[b,j]
    out[a,i] = sum_{b,h} relu(U[b,h]+b1[h]-U[a,h]) * G[b,h,i]
             + sum_j b2[i,j] * (sum_b f[b,j])

Structure follows the HW-proven DRAM-bounce regroup for G, with:
  - PSUM->SBUF copies of G\' split across DVE and ACT; the bounce DMA and
    the gathers issue from SP where sentinel nops absorb extra sync waits
    (SP SEQ is in-order, so a wait on an earlier nop still happens-before).
  - negua2 in bf16 so DVE T-builds hit the 4x perf mode; T engine split
    rebalanced DVE-heavy.
  - Main contraction in out[a,i] layout: per b-pair two accumulating
    matmuls with lhsT = T_p a-half (128 wide) and rhs = g_sb[:,p,:]
    (32 cols) -> 64 matmuls of ~13ns instead of 32 of ~107ns.
  - b2 term via lhsT = s_bcast half (materialized fsum broadcast).
"""

import sys

_TRN_REPO = "/opt/trn_rl_repo"
if _TRN_REPO not in sys.path:
    sys.path.insert(0, _TRN_REPO)

from contextlib import ExitStack

import numpy as np

import concourse.bass as bass
import concourse.mybir as mybir
import concourse.tile as tile
from concourse.bass_utils import run_bass_kernel_spmd

from concourse.vector_clock import ScopedClock

_orig_drain_and_barrier = tile.TileContext._drain_and_barrier


def _split_wait_drain_and_barrier(self, tick_clock, wait_clock):
    nc = self.nc
    probe = nc.sync.nop(nofuse=True)
    wait_clock.add_sem_waits(probe.ins, ScopedClock({None: tick_clock.global_clock}))
    si = probe.ins.sync_info
    waits = list(si.on_wait) if si is not None and si.on_wait else []
    if len(waits) > 1:
        probe.ins.sync_info = mybir.SyncInfo(on_wait=waits[:1], on_update=[])
        for w in waits[1:]:
            extra = nc.sync.nop(nofuse=True)
            extra.ins.sync_info = mybir.SyncInfo(on_wait=[w], on_update=[])
    nc.sync.drain()
    nc.all_engine_barrier()
    popped = nc._tile_sem_poison_stack.pop()
    assert popped is self._sem_poison
    nc.clear_and_free_semaphores(list(self.sems.allocated().values()))
    nc.all_engine_barrier()


tile.TileContext._drain_and_barrier = _split_wait_drain_and_barrier

F32 = mybir.dt.float32
BF16 = mybir.dt.bfloat16
Z, N, C_IN, C_OUT, H = 2, 256, 32, 32, 64
BQ = 64
NPAIR = BQ // 2

PKW = 289
MPW = 2496
MA = 1472

T_ENGINES = (
    ["g", "v", "v", "s", "v", "g", "v", "v",
     "s", "v", "g", "v", "v", "s", "v", "g",
     "v", "s", "v", "g", "v", "v", "s", "v",
     "g", "v", "s", "v", "g", "v", "v", "v"]
)
N_WARMERS = 16


def build_nc(debug: bool = False) -> bass.Bass:
    nc = bass.Bass("TRN2", target_bir_lowering=False, debug=debug, num_devices=8)

    m2p = nc.dram_tensor("M2p", [C_IN, MPW], BF16, kind="ExternalInput").ap()
    pk = nc.dram_tensor("pk", [H, PKW], F32, kind="ExternalInput").ap()
    outp = nc.dram_tensor("outp", [N, C_OUT], F32, kind="ExternalOutput").ap()

    with tile.TileContext(nc) as tc, ExitStack() as ctx:
        consts = ctx.enter_context(tc.tile_pool(name="consts", bufs=1))
        work = ctx.enter_context(tc.tile_pool(name="work", bufs=1))
        tpool = ctx.enter_context(tc.tile_pool(name="tpool", bufs=NPAIR))
        psum = ctx.enter_context(tc.tile_pool(name="psum", bufs=1, space="PSUM"))
        dpool = ctx.enter_context(tc.tile_pool(name="dpool", bufs=1, space="DRAM"))

        m2p_sb = consts.tile([C_IN, MPW], BF16)
        nc.sync.dma_start(out=m2p_sb[:, 0:MA], in_=m2p[:, 0:MA])
        nc.sync.dma_start(out=m2p_sb[:, MA:MPW], in_=m2p[:, MA:MPW])
        pk_sb = consts.tile([H, PKW], F32)
        nc.gpsimd.dma_start(out=pk_sb, in_=pk)

        fTq_bf = m2p_sb[:, 0:64]
        gT_bf = m2p_sb[0:3, 64:320]
        gTb_bf = m2p_sb[0:3, 320:384]
        w1_bf = m2p_sb[0:3, 384:448]
        fTfull_sb = pk_sb[0:C_IN, 0:256]
        b2t_sb = pk_sb[0:C_IN, 256:288]
        b1_sb = pk_sb[0:H, 288:289]

        scol = work.tile([C_IN, 1], F32)
        nc.vector.tensor_reduce(out=scol, in_=fTfull_sb,
                                axis=mybir.AxisListType.X, op=mybir.AluOpType.add)
        s_bcast = work.tile([C_IN, N], BF16)
        nc.vector.tensor_scalar(out=s_bcast, in0=scol.broadcast_to([C_IN, N]),
                                scalar1=0.0, scalar2=None,
                                op0=mybir.AluOpType.add)
        b2t_bf = work.tile([C_IN, C_OUT], BF16)
        nc.vector.tensor_copy(b2t_bf, b2t_sb)

        u_ps = psum.tile([H, N + BQ], F32)
        uaT_ps = u_ps[:, 0:N]
        ubT_ps = u_ps[:, N:N + BQ]
        nc.tensor.matmul(uaT_ps, lhsT=w1_bf, rhs=gT_bf, start=True, stop=True)
        nc.tensor.matmul(ubT_ps, lhsT=w1_bf, rhs=gTb_bf, start=True, stop=True)

        tc.tile_set_cur_wait(0.006)
        negua2 = work.tile([2 * H, N], BF16)
        nc.scalar.activation(negua2[0:H, :], uaT_ps,
                             mybir.ActivationFunctionType.Copy, scale=-1.0)
        nc.scalar.activation(negua2[H:2 * H, :], uaT_ps,
                             mybir.ActivationFunctionType.Copy, scale=-1.0)

        ubB = work.tile([H, BQ], F32)
        nc.vector.tensor_scalar(out=ubB, in0=ubT_ps, scalar1=b1_sb,
                                scalar2=None, op0=mybir.AluOpType.add)
        ubT2 = work.tile([2 * H, NPAIR], F32)
        ubB_r = ubB.rearrange("h (p two) -> h two p", two=2)
        nc.scalar.activation(ubT2[0:H, :], ubB_r[:, 0, :],
                             mybir.ActivationFunctionType.Copy)
        nc.scalar.activation(ubT2[H:2 * H, :], ubB_r[:, 1, :],
                             mybir.ActivationFunctionType.Copy)

        tc.tile_set_cur_wait(0.0)
        g_ps = []
        for k in range(4):
            gp = psum.tile([BQ, 512], F32, name=f"g_ps{k}", tag=f"g_ps{k}")
            nc.tensor.matmul(gp, lhsT=fTq_bf,
                             rhs=m2p_sb[:, 448 + k * 512:448 + (k + 1) * 512],
                             start=True, stop=True)
            g_ps.append(gp)

        g_tmp = work.tile([BQ, H * C_OUT], BF16)
        for k, eng in ((0, "v"), (1, "s"), (2, "s"), (3, "v")):
            dst = g_tmp[:, k * 512:(k + 1) * 512]
            if eng == "v":
                nc.vector.tensor_copy(dst, g_ps[k])
            else:
                nc.scalar.activation(dst, g_ps[k],
                                     mybir.ActivationFunctionType.Copy)

        g_sb = work.tile([2 * H, NPAIR, C_OUT], BF16)
        g_dram = dpool.tile([BQ, H * C_OUT], BF16)
        sponges = [nc.sync.nop(nofuse=True).ins for _ in range(4)]
        nc.sync.dma_start(out=g_dram, in_=g_tmp)
        g0 = g_dram[:, :]
        for ph in range(2):
            g_src = bass.AP(tensor=g0.tensor,
                            offset=g0.offset + ph * 16 * 4096,
                            ap=[[32, 2 * H], [4096, 16], [1, C_OUT]])
            nc.sync.dma_start(out=g_sb[:, 16 * ph:16 * (ph + 1), :],
                              in_=g_src)

        acc = psum.tile([128, 2 * C_OUT], F32)
        nc.tensor.matmul(acc[:, 0:C_OUT], lhsT=s_bcast[:, 0:128], rhs=b2t_bf,
                         start=True, stop=False)
        nc.tensor.matmul(acc[:, C_OUT:2 * C_OUT], lhsT=s_bcast[:, 128:256],
                         rhs=b2t_bf, start=True, stop=False)

        scrap = psum.tile([C_OUT, 1], F32)

        def observe_gather(ph):
            nc.tensor.matmul(scrap, lhsT=g_sb[:, 16 * ph, :],
                             rhs=g_sb[:, 16 * ph, 0:1],
                             start=True, stop=True)

        warm_ps = psum.tile([C_OUT, N], F32)
        t_tiles = []
        for p in range(NPAIR):
            t_p = tpool.tile([2 * H, N], BF16, tag="T", name=f"t_{p}")
            t_tiles.append(t_p)
            eng = T_ENGINES[p]
            if eng == "s":
                nc.scalar.activation(t_p, negua2,
                                     mybir.ActivationFunctionType.Relu,
                                     bias=ubT2[:, p:p + 1], scale=1.0)
            else:
                e = nc.vector if eng == "v" else nc.gpsimd
                e.tensor_scalar(out=t_p, in0=negua2,
                                scalar1=ubT2[:, p:p + 1], scalar2=0.0,
                                op0=mybir.AluOpType.add,
                                op1=mybir.AluOpType.max)
            if p < N_WARMERS:
                nc.tensor.matmul(warm_ps, lhsT=t_p[0:C_IN, 0:C_OUT],
                                 rhs=t_p[0:C_IN, :], start=True, stop=True)
        for ph in range(2):
            observe_gather(ph)
            for p in range(16 * ph, 16 * (ph + 1)):
                rhs = g_sb[:, p, :]
                last = p == NPAIR - 1
                nc.tensor.matmul(acc[:, 0:C_OUT], lhsT=t_tiles[p][:, 0:128],
                                 rhs=rhs, start=False, stop=last)
                nc.tensor.matmul(acc[:, C_OUT:2 * C_OUT],
                                 lhsT=t_tiles[p][:, 128:256],
                                 rhs=rhs, start=False, stop=last)

        out_sb = work.tile([128, 2 * C_OUT], F32)
        nc.vector.tensor_copy(out_sb[:, 0:C_OUT], acc[:, 0:C_OUT])
        nc.vector.tensor_copy(out_sb[:, C_OUT:2 * C_OUT],
                              acc[:, C_OUT:2 * C_OUT])
        sponges += [nc.sync.nop(nofuse=True).ins for _ in range(2)]
        dst = bass.AP(tensor=outp.tensor, offset=outp.offset,
                      ap=[[C_OUT, 128], [128 * C_OUT, 2], [1, C_OUT]])
        nc.sync.dma_start(out=dst,
                          in_=out_sb.rearrange("p (h i) -> p h i", i=C_OUT))

    sponge_names = {sp.name for sp in sponges}
    for blk in nc.m.functions[0].blocks:
        insts = list(blk.instructions)
        for idx, ins in enumerate(insts):
            si = ins.sync_info
            if si is None or not si.on_wait or len(si.on_wait) <= 1:
                continue
            waits = list(si.on_wait)
            assert str(ins.engine).endswith("SP"), (
                f"multi-wait on non-SP instruction {ins.name} "
                f"{ins.engine} {ins.opcode}: "
                f"{[(w.ant_name, w.wait_value) for w in waits]}"
            )
            extras, keep = waits[:-1], waits[-1:]
            j = idx - 1
            while extras and j >= 0:
                prev = insts[j]
                j -= 1
                if prev.name not in sponge_names:
                    continue
                psi = prev.sync_info
                if psi is not None and psi.on_wait:
                    continue
                prev.sync_info = mybir.SyncInfo(on_wait=[extras.pop()],
                                                on_update=[])
            assert not extras, f"no sponge for {ins.name}"
            ins.sync_info = mybir.SyncInfo(on_wait=keep,
                                           on_update=list(si.on_update or []))
    return nc


def shard_inputs(features, geometry, W1, b1, W2, b2) -> list[dict]:
    import ml_dtypes
    bf16 = ml_dtypes.bfloat16
    f = np.ascontiguousarray(np.asarray(features, np.float32))
    g = np.ascontiguousarray(np.asarray(geometry, np.float32))
    W1 = np.ascontiguousarray(np.asarray(W1, np.float32))
    b1 = np.ascontiguousarray(np.asarray(b1, np.float32))
    W2 = np.ascontiguousarray(np.asarray(W2, np.float32))
    b2 = np.ascontiguousarray(np.asarray(b2, np.float32))

    m2p = W2.reshape(H, C_OUT, C_IN).transpose(2, 0, 1).reshape(C_IN, H * C_OUT)
    b2t = np.ascontiguousarray(b2.reshape(C_OUT, C_IN).T)

    maps = []
    for core in range(8):
        z, q = divmod(core, 4)
        sl = slice(q * BQ, (q + 1) * BQ)
        pkv = np.zeros((H, PKW), np.float32)
        pkv[0:C_IN, 0:256] = f[z].T
        if q == 0:
            pkv[0:C_IN, 256:288] = b2t
        pkv[0:H, 288] = b1
        mp = np.zeros((C_IN, MPW), bf16)
        mp[:, 0:64] = f[z, sl].T.astype(bf16)
        mp[0:3, 64:320] = g[z].T.astype(bf16)
        mp[0:3, 320:384] = g[z, sl].T.astype(bf16)
        mp[0:3, 384:448] = W1.astype(bf16)
        mp[:, 448:2496] = m2p.astype(bf16)
        maps.append({"pk": pkv, "M2p": mp})
    return maps


def unshard(parts: list[np.ndarray]) -> np.ndarray:
    out = np.empty((Z, N, C_OUT), np.float32)
    for z in range(Z):
        acc = parts[4 * z].astype(np.float32)
        for q in range(1, 4):
            acc = acc + parts[4 * z + q]
        out[z] = acc
    return out


def kernel(**inputs) -> np.ndarray:
    nc = build_nc(debug=False)
    in_maps = shard_inputs(**inputs)
    res = run_bass_kernel_spmd(nc, in_maps, list(range(8)))
    return unshard([r["outp"] for r in res.results])


# revision 30
# speedup vs baseline: 1.2090x; 1.0105x over previous
"""Trainium2 Bass kernel for the pairwise-MLP geometric convolution.

Factorization (per core: one z, one b-quarter of 64 points):
    U = g @ W1;  G[b,h,i] = sum_j W2[h, i*C_IN+j] f[b,j]
    out[a,i] = sum_{b,h} relu(U[b,h]+b1[h]-U[a,h]) * G[b,h,i]
             + sum_j b2[i,j] * (sum_b f[b,j])

Structure follows the HW-proven DRAM-bounce regroup for G, with:
  - PSUM->SBUF copies of G\' split across DVE and ACT; the bounce DMA and
    the gathers issue from SP where sentinel nops absorb extra sync waits
    (SP SEQ is in-order, so a wait on an earlier nop still happens-before).
  - negua2 in bf16 so DVE T-builds hit the 4x perf mode; T engine split
    rebalanced DVE-heavy.
  - Main contraction in out[a,i] layout: per b-pair two accumulating
    matmuls with lhsT = T_p a-half (128 wide) and rhs = g_sb[:,p,:]
    (32 cols) -> 64 matmuls of ~13ns instead of 32 of ~107ns.
  - b2 term via lhsT = s_bcast half (materialized fsum broadcast).
"""

import sys

_TRN_REPO = "/opt/trn_rl_repo"
if _TRN_REPO not in sys.path:
    sys.path.insert(0, _TRN_REPO)

from contextlib import ExitStack

import numpy as np

import concourse.bass as bass
import concourse.mybir as mybir
import concourse.tile as tile
from concourse.bass_utils import run_bass_kernel_spmd

from concourse.vector_clock import ScopedClock

_orig_drain_and_barrier = tile.TileContext._drain_and_barrier


def _split_wait_drain_and_barrier(self, tick_clock, wait_clock):
    nc = self.nc
    probe = nc.sync.nop(nofuse=True)
    wait_clock.add_sem_waits(probe.ins, ScopedClock({None: tick_clock.global_clock}))
    si = probe.ins.sync_info
    waits = list(si.on_wait) if si is not None and si.on_wait else []
    if len(waits) > 1:
        probe.ins.sync_info = mybir.SyncInfo(on_wait=waits[:1], on_update=[])
        for w in waits[1:]:
            extra = nc.sync.nop(nofuse=True)
            extra.ins.sync_info = mybir.SyncInfo(on_wait=[w], on_update=[])
    nc.sync.drain()
    nc.all_engine_barrier()
    popped = nc._tile_sem_poison_stack.pop()
    assert popped is self._sem_poison
    nc.clear_and_free_semaphores(list(self.sems.allocated().values()))
    nc.all_engine_barrier()


tile.TileContext._drain_and_barrier = _split_wait_drain_and_barrier

F32 = mybir.dt.float32
BF16 = mybir.dt.bfloat16
Z, N, C_IN, C_OUT, H = 2, 256, 32, 32, 64
BQ = 64
NPAIR = BQ // 2

PKW = 289
MPW = 2496
MA = 1472

T_ENGINES = (
    ["g", "v", "v", "s", "v", "g", "v", "v",
     "s", "v", "g", "v", "v", "s", "v", "g",
     "v", "s", "v", "g", "v", "v", "s", "v",
     "g", "v", "s", "v", "g", "v", "v", "v"]
)
N_WARMERS = 16


def build_nc(debug: bool = False) -> bass.Bass:
    nc = bass.Bass("TRN2", target_bir_lowering=False, debug=debug, num_devices=8)

    m2p = nc.dram_tensor("M2p", [C_IN, MPW], BF16, kind="ExternalInput").ap()
    pk = nc.dram_tensor("pk", [H, PKW], F32, kind="ExternalInput").ap()
    outp = nc.dram_tensor("outp", [N, C_OUT], F32, kind="ExternalOutput").ap()

    with tile.TileContext(nc) as tc, ExitStack() as ctx:
        consts = ctx.enter_context(tc.tile_pool(name="consts", bufs=1))
        work = ctx.enter_context(tc.tile_pool(name="work", bufs=1))
        tpool = ctx.enter_context(tc.tile_pool(name="tpool", bufs=NPAIR))
        psum = ctx.enter_context(tc.tile_pool(name="psum", bufs=1, space="PSUM"))
        dpool = ctx.enter_context(tc.tile_pool(name="dpool", bufs=1, space="DRAM"))

        m2p_sb = consts.tile([C_IN, MPW], BF16)
        nc.sync.dma_start(out=m2p_sb[:, 0:MA], in_=m2p[:, 0:MA])
        nc.sync.dma_start(out=m2p_sb[:, MA:MPW], in_=m2p[:, MA:MPW])
        pk_sb = consts.tile([H, PKW], F32)
        nc.gpsimd.dma_start(out=pk_sb, in_=pk)

        fTq_bf = m2p_sb[:, 0:64]
        gT_bf = m2p_sb[0:3, 64:320]
        gTb_bf = m2p_sb[0:3, 320:384]
        w1_bf = m2p_sb[0:3, 384:448]
        fTfull_sb = pk_sb[0:C_IN, 0:256]
        b2t_sb = pk_sb[0:C_IN, 256:288]
        b1_sb = pk_sb[0:H, 288:289]

        scol = work.tile([C_IN, 1], F32)
        nc.vector.tensor_reduce(out=scol, in_=fTfull_sb,
                                axis=mybir.AxisListType.X, op=mybir.AluOpType.add)
        tc.tile_set_cur_wait(0.006)
        s_bcast = work.tile([C_IN, N], BF16)
        nc.vector.tensor_scalar(out=s_bcast, in0=scol.broadcast_to([C_IN, N]),
                                scalar1=0.0, scalar2=None,
                                op0=mybir.AluOpType.add)
        b2t_bf = work.tile([C_IN, C_OUT], BF16)
        nc.vector.tensor_copy(b2t_bf, b2t_sb)

        tc.tile_set_cur_wait(0.0)
        u_ps = psum.tile([H, N + BQ], F32)
        uaT_ps = u_ps[:, 0:N]
        ubT_ps = u_ps[:, N:N + BQ]
        nc.tensor.matmul(uaT_ps, lhsT=w1_bf, rhs=gT_bf, start=True, stop=True)
        nc.tensor.matmul(ubT_ps, lhsT=w1_bf, rhs=gTb_bf, start=True, stop=True)

        tc.tile_set_cur_wait(0.006)
        negua2 = work.tile([2 * H, N], BF16)
        nc.scalar.activation(negua2[0:H, :], uaT_ps,
                             mybir.ActivationFunctionType.Copy, scale=-1.0)
        nc.scalar.activation(negua2[H:2 * H, :], uaT_ps,
                             mybir.ActivationFunctionType.Copy, scale=-1.0)

        ubB = work.tile([H, BQ], F32)
        nc.vector.tensor_scalar(out=ubB, in0=ubT_ps, scalar1=b1_sb,
                                scalar2=None, op0=mybir.AluOpType.add)
        ubT2 = work.tile([2 * H, NPAIR], F32)
        ubB_r = ubB.rearrange("h (p two) -> h two p", two=2)
        nc.scalar.activation(ubT2[0:H, :], ubB_r[:, 0, :],
                             mybir.ActivationFunctionType.Copy)
        nc.scalar.activation(ubT2[H:2 * H, :], ubB_r[:, 1, :],
                             mybir.ActivationFunctionType.Copy)

        tc.tile_set_cur_wait(0.0)
        g_ps = []
        for k in range(4):
            gp = psum.tile([BQ, 512], F32, name=f"g_ps{k}", tag=f"g_ps{k}")
            nc.tensor.matmul(gp, lhsT=fTq_bf,
                             rhs=m2p_sb[:, 448 + k * 512:448 + (k + 1) * 512],
                             start=True, stop=True)
            g_ps.append(gp)

        g_tmp = work.tile([BQ, H * C_OUT], BF16)
        for k, eng in ((0, "v"), (1, "s"), (2, "s"), (3, "v")):
            dst = g_tmp[:, k * 512:(k + 1) * 512]
            if eng == "v":
                nc.vector.tensor_copy(dst, g_ps[k])
            else:
                nc.scalar.activation(dst, g_ps[k],
                                     mybir.ActivationFunctionType.Copy)

        g_sb = work.tile([2 * H, NPAIR, C_OUT], BF16)
        g_dram = dpool.tile([BQ, H * C_OUT], BF16)
        sponges = [nc.sync.nop(nofuse=True).ins for _ in range(4)]
        nc.sync.dma_start(out=g_dram, in_=g_tmp)
        g0 = g_dram[:, :]
        for ph in range(2):
            g_src = bass.AP(tensor=g0.tensor,
                            offset=g0.offset + ph * 16 * 4096,
                            ap=[[32, 2 * H], [4096, 16], [1, C_OUT]])
            nc.sync.dma_start(out=g_sb[:, 16 * ph:16 * (ph + 1), :],
                              in_=g_src)

        acc = psum.tile([128, 2 * C_OUT], F32)
        nc.tensor.matmul(acc[:, 0:C_OUT], lhsT=s_bcast[:, 0:128], rhs=b2t_bf,
                         start=True, stop=False)
        nc.tensor.matmul(acc[:, C_OUT:2 * C_OUT], lhsT=s_bcast[:, 128:256],
                         rhs=b2t_bf, start=True, stop=False)

        scrap = psum.tile([C_OUT, 1], F32)

        def observe_gather(ph):
            nc.tensor.matmul(scrap, lhsT=g_sb[:, 16 * ph, :],
                             rhs=g_sb[:, 16 * ph, 0:1],
                             start=True, stop=True)

        warm_ps = psum.tile([C_OUT, N], F32)
        t_tiles = []
        for p in range(NPAIR):
            t_p = tpool.tile([2 * H, N], BF16, tag="T", name=f"t_{p}")
            t_tiles.append(t_p)
            eng = T_ENGINES[p]
            if eng == "s":
                nc.scalar.activation(t_p, negua2,
                                     mybir.ActivationFunctionType.Relu,
                                     bias=ubT2[:, p:p + 1], scale=1.0)
            else:
                e = nc.vector if eng == "v" else nc.gpsimd
                e.tensor_scalar(out=t_p, in0=negua2,
                                scalar1=ubT2[:, p:p + 1], scalar2=0.0,
                                op0=mybir.AluOpType.add,
                                op1=mybir.AluOpType.max)
            if p < N_WARMERS:
                nc.tensor.matmul(warm_ps, lhsT=t_p[0:C_IN, 0:C_OUT],
                                 rhs=t_p[0:C_IN, :], start=True, stop=True)
        for ph in range(2):
            observe_gather(ph)
            for p in range(16 * ph, 16 * (ph + 1)):
                rhs = g_sb[:, p, :]
                last = p == NPAIR - 1
                nc.tensor.matmul(acc[:, 0:C_OUT], lhsT=t_tiles[p][:, 0:128],
                                 rhs=rhs, start=False, stop=last)
                nc.tensor.matmul(acc[:, C_OUT:2 * C_OUT],
                                 lhsT=t_tiles[p][:, 128:256],
                                 rhs=rhs, start=False, stop=last)

        out_sb = work.tile([128, 2 * C_OUT], F32)
        nc.vector.tensor_copy(out_sb[:, 0:C_OUT], acc[:, 0:C_OUT])
        nc.vector.tensor_copy(out_sb[:, C_OUT:2 * C_OUT],
                              acc[:, C_OUT:2 * C_OUT])
        sponges += [nc.sync.nop(nofuse=True).ins for _ in range(2)]
        dst = bass.AP(tensor=outp.tensor, offset=outp.offset,
                      ap=[[C_OUT, 128], [128 * C_OUT, 2], [1, C_OUT]])
        nc.sync.dma_start(out=dst,
                          in_=out_sb.rearrange("p (h i) -> p h i", i=C_OUT))

    sponge_names = {sp.name for sp in sponges}
    for blk in nc.m.functions[0].blocks:
        insts = list(blk.instructions)
        for idx, ins in enumerate(insts):
            si = ins.sync_info
            if si is None or not si.on_wait or len(si.on_wait) <= 1:
                continue
            waits = list(si.on_wait)
            assert str(ins.engine).endswith("SP"), (
                f"multi-wait on non-SP instruction {ins.name} "
                f"{ins.engine} {ins.opcode}: "
                f"{[(w.ant_name, w.wait_value) for w in waits]}"
            )
            extras, keep = waits[:-1], waits[-1:]
            j = idx - 1
            while extras and j >= 0:
                prev = insts[j]
                j -= 1
                if prev.name not in sponge_names:
                    continue
                psi = prev.sync_info
                if psi is not None and psi.on_wait:
                    continue
                prev.sync_info = mybir.SyncInfo(on_wait=[extras.pop()],
                                                on_update=[])
            assert not extras, f"no sponge for {ins.name}"
            ins.sync_info = mybir.SyncInfo(on_wait=keep,
                                           on_update=list(si.on_update or []))
    return nc


def shard_inputs(features, geometry, W1, b1, W2, b2) -> list[dict]:
    import ml_dtypes
    bf16 = ml_dtypes.bfloat16
    f = np.ascontiguousarray(np.asarray(features, np.float32))
    g = np.ascontiguousarray(np.asarray(geometry, np.float32))
    W1 = np.ascontiguousarray(np.asarray(W1, np.float32))
    b1 = np.ascontiguousarray(np.asarray(b1, np.float32))
    W2 = np.ascontiguousarray(np.asarray(W2, np.float32))
    b2 = np.ascontiguousarray(np.asarray(b2, np.float32))

    m2p = W2.reshape(H, C_OUT, C_IN).transpose(2, 0, 1).reshape(C_IN, H * C_OUT)
    b2t = np.ascontiguousarray(b2.reshape(C_OUT, C_IN).T)

    maps = []
    for core in range(8):
        z, q = divmod(core, 4)
        sl = slice(q * BQ, (q + 1) * BQ)
        pkv = np.zeros((H, PKW), np.float32)
        pkv[0:C_IN, 0:256] = f[z].T
        if q == 0:
            pkv[0:C_IN, 256:288] = b2t
        pkv[0:H, 288] = b1
        mp = np.zeros((C_IN, MPW), bf16)
        mp[:, 0:64] = f[z, sl].T.astype(bf16)
        mp[0:3, 64:320] = g[z].T.astype(bf16)
        mp[0:3, 320:384] = g[z, sl].T.astype(bf16)
        mp[0:3, 384:448] = W1.astype(bf16)
        mp[:, 448:2496] = m2p.astype(bf16)
        maps.append({"pk": pkv, "M2p": mp})
    return maps


def unshard(parts: list[np.ndarray]) -> np.ndarray:
    out = np.empty((Z, N, C_OUT), np.float32)
    for z in range(Z):
        acc = parts[4 * z].astype(np.float32)
        for q in range(1, 4):
            acc = acc + parts[4 * z + q]
        out[z] = acc
    return out


def kernel(**inputs) -> np.ndarray:
    nc = build_nc(debug=False)
    in_maps = shard_inputs(**inputs)
    res = run_bass_kernel_spmd(nc, in_maps, list(range(8)))
    return unshard([r["outp"] for r in res.results])


# revision 37
# speedup vs baseline: 1.2298x; 1.0172x over previous
"""Trainium2 Bass kernel for the pairwise-MLP geometric convolution.

Factorization (per core: one z, one b-quarter of 64 points):
    U = g @ W1;  G[b,h,i] = sum_j W2[h, i*C_IN+j] f[b,j]
    out[a,i] = sum_{b,h} relu(U[b,h]+b1[h]-U[a,h]) * G[b,h,i]
             + sum_j b2[i,j] * (sum_b f[b,j])

Structure follows the HW-proven DRAM-bounce regroup for G, with:
  - PSUM->SBUF copies of G\' split across DVE and ACT; the bounce DMA and
    the gathers issue from SP where sentinel nops absorb extra sync waits
    (SP SEQ is in-order, so a wait on an earlier nop still happens-before).
  - negua2 in bf16 so DVE T-builds hit the 4x perf mode; T engine split
    rebalanced DVE-heavy.
  - Main contraction in out[a,i] layout: per b-pair two accumulating
    matmuls with lhsT = T_p a-half (128 wide) and rhs = g_sb[:,p,:]
    (32 cols) -> 64 matmuls of ~13ns instead of 32 of ~107ns.
  - b2 term via lhsT = s_bcast half (materialized fsum broadcast).
"""

import sys

_TRN_REPO = "/opt/trn_rl_repo"
if _TRN_REPO not in sys.path:
    sys.path.insert(0, _TRN_REPO)

from contextlib import ExitStack

import numpy as np

import concourse.bass as bass
import concourse.mybir as mybir
import concourse.tile as tile
from concourse.bass_utils import run_bass_kernel_spmd

from concourse.vector_clock import ScopedClock

_orig_drain_and_barrier = tile.TileContext._drain_and_barrier


def _split_wait_drain_and_barrier(self, tick_clock, wait_clock):
    nc = self.nc
    probe = nc.sync.nop(nofuse=True)
    wait_clock.add_sem_waits(probe.ins, ScopedClock({None: tick_clock.global_clock}))
    si = probe.ins.sync_info
    waits = list(si.on_wait) if si is not None and si.on_wait else []
    if len(waits) > 1:
        probe.ins.sync_info = mybir.SyncInfo(on_wait=waits[:1], on_update=[])
        for w in waits[1:]:
            extra = nc.sync.nop(nofuse=True)
            extra.ins.sync_info = mybir.SyncInfo(on_wait=[w], on_update=[])
    nc.sync.drain()
    nc.all_engine_barrier()
    popped = nc._tile_sem_poison_stack.pop()
    assert popped is self._sem_poison
    nc.clear_and_free_semaphores(list(self.sems.allocated().values()))
    nc.all_engine_barrier()


tile.TileContext._drain_and_barrier = _split_wait_drain_and_barrier

F32 = mybir.dt.float32
BF16 = mybir.dt.bfloat16
Z, N, C_IN, C_OUT, H = 2, 256, 32, 32, 64
BQ = 64
NPAIR = BQ // 2

PKW = 289
MPW = 2496
MA = 1472

T_ENGINES = (
    ["g", "v", "v", "s", "v", "g", "v", "v",
     "s", "v", "g", "v", "v", "s", "v", "g",
     "v", "s", "v", "g", "v", "v", "s", "v",
     "g", "v", "s", "v", "g", "v", "v", "v"]
)
N_WARMERS = 16


def build_nc(debug: bool = False) -> bass.Bass:
    nc = bass.Bass("TRN2", target_bir_lowering=False, debug=debug, num_devices=8)

    m2p = nc.dram_tensor("M2p", [C_IN, MPW], BF16, kind="ExternalInput").ap()
    pk = nc.dram_tensor("pk", [H, PKW], F32, kind="ExternalInput").ap()
    outp = nc.dram_tensor("outp", [N, C_OUT], F32, kind="ExternalOutput").ap()

    with tile.TileContext(nc) as tc, ExitStack() as ctx:
        consts = ctx.enter_context(tc.tile_pool(name="consts", bufs=1))
        work = ctx.enter_context(tc.tile_pool(name="work", bufs=1))
        tpool = ctx.enter_context(tc.tile_pool(name="tpool", bufs=NPAIR))
        psum = ctx.enter_context(tc.tile_pool(name="psum", bufs=1, space="PSUM"))
        dpool = ctx.enter_context(tc.tile_pool(name="dpool", bufs=1, space="DRAM"))

        m2p_sb = consts.tile([C_IN, MPW], BF16)
        nc.sync.dma_start(out=m2p_sb[:, 0:MA], in_=m2p[:, 0:MA])
        nc.sync.dma_start(out=m2p_sb[:, MA:MPW], in_=m2p[:, MA:MPW])
        pk_sb = consts.tile([H, PKW], F32)
        nc.gpsimd.dma_start(out=pk_sb, in_=pk)

        fTq_bf = m2p_sb[:, 0:64]
        gT_bf = m2p_sb[0:3, 64:320]
        gTb_bf = m2p_sb[0:3, 320:384]
        w1_bf = m2p_sb[0:3, 384:448]
        fTfull_sb = pk_sb[0:C_IN, 0:256]
        b2t_sb = pk_sb[0:C_IN, 256:288]
        b1_sb = pk_sb[0:H, 288:289]

        scol = work.tile([C_IN, 1], F32)
        nc.vector.tensor_reduce(out=scol, in_=fTfull_sb,
                                axis=mybir.AxisListType.X, op=mybir.AluOpType.add)
        tc.tile_set_cur_wait(0.006)
        s_bcast = work.tile([C_IN, N], BF16)
        nc.vector.tensor_scalar(out=s_bcast, in0=scol.broadcast_to([C_IN, N]),
                                scalar1=0.0, scalar2=None,
                                op0=mybir.AluOpType.add)
        b2t_bf = work.tile([C_IN, C_OUT], BF16)
        nc.vector.tensor_copy(b2t_bf, b2t_sb)

        tc.tile_set_cur_wait(0.0)
        u_ps = psum.tile([H, N + BQ], F32)
        uaT_ps = u_ps[:, 0:N]
        ubT_ps = u_ps[:, N:N + BQ]
        nc.tensor.matmul(uaT_ps, lhsT=w1_bf, rhs=gT_bf, start=True, stop=True)
        nc.tensor.matmul(ubT_ps, lhsT=w1_bf, rhs=gTb_bf, start=True, stop=True)

        tc.tile_set_cur_wait(0.006)
        negua2 = work.tile([2 * H, N], BF16)
        nc.scalar.activation(negua2[0:H, :], uaT_ps,
                             mybir.ActivationFunctionType.Copy, scale=-1.0)
        nc.scalar.activation(negua2[H:2 * H, :], uaT_ps,
                             mybir.ActivationFunctionType.Copy, scale=-1.0)

        ubB = work.tile([H, BQ], F32)
        nc.vector.tensor_scalar(out=ubB, in0=ubT_ps, scalar1=b1_sb,
                                scalar2=None, op0=mybir.AluOpType.add)
        ubT2 = work.tile([2 * H, NPAIR], F32)
        ubB_r = ubB.rearrange("h (p two) -> h two p", two=2)
        nc.scalar.activation(ubT2[0:H, :], ubB_r[:, 0, :],
                             mybir.ActivationFunctionType.Copy)
        nc.scalar.activation(ubT2[H:2 * H, :], ubB_r[:, 1, :],
                             mybir.ActivationFunctionType.Copy)

        tc.tile_set_cur_wait(0.0)
        g_ps = []
        for k in range(4):
            gp = psum.tile([BQ, 512], F32, name=f"g_ps{k}", tag=f"g_ps{k}")
            nc.tensor.matmul(gp, lhsT=fTq_bf,
                             rhs=m2p_sb[:, 448 + k * 512:448 + (k + 1) * 512],
                             start=True, stop=True)
            g_ps.append(gp)

        g_tmp = work.tile([BQ, H * C_OUT], BF16)
        for k, eng in ((0, "v"), (1, "s"), (2, "s"), (3, "v")):
            dst = g_tmp[:, k * 512:(k + 1) * 512]
            if eng == "v":
                nc.vector.tensor_copy(dst, g_ps[k])
            else:
                nc.scalar.activation(dst, g_ps[k],
                                     mybir.ActivationFunctionType.Copy)

        g_sb = work.tile([2 * H, NPAIR, C_OUT], BF16)
        g_dram = dpool.tile([BQ, H * C_OUT], BF16)
        sponges = [nc.sync.nop(nofuse=True).ins for _ in range(4)]
        nc.sync.dma_start(out=g_dram, in_=g_tmp)
        g0 = g_dram[:, :]
        for p0, p1 in ((0, 16), (16, 32)):
            g_src = bass.AP(tensor=g0.tensor,
                            offset=g0.offset + p0 * 4096,
                            ap=[[32, 2 * H], [4096, p1 - p0], [1, C_OUT]])
            nc.sync.dma_start(out=g_sb[:, p0:p1, :], in_=g_src)

        acc = psum.tile([128, 2 * C_OUT], F32)
        nc.tensor.matmul(acc[:, 0:C_OUT], lhsT=s_bcast[:, 0:128], rhs=b2t_bf,
                         start=True, stop=False)
        nc.tensor.matmul(acc[:, C_OUT:2 * C_OUT], lhsT=s_bcast[:, 128:256],
                         rhs=b2t_bf, start=True, stop=False)

        scrap = psum.tile([C_OUT, 1], F32)

        def observe_gather(ph):
            pp = 0 if ph == 0 else 16
            nc.tensor.matmul(scrap, lhsT=g_sb[:, pp, :],
                             rhs=g_sb[:, pp, 0:1],
                             start=True, stop=True)

        warm_ps = psum.tile([C_OUT, N], F32)
        t_tiles = []
        for p in range(NPAIR):
            t_p = tpool.tile([2 * H, N], BF16, tag="T", name=f"t_{p}")
            t_tiles.append(t_p)
            eng = T_ENGINES[p]
            if eng == "s":
                nc.scalar.activation(t_p, negua2,
                                     mybir.ActivationFunctionType.Relu,
                                     bias=ubT2[:, p:p + 1], scale=1.0)
            else:
                e = nc.vector if eng == "v" else nc.gpsimd
                e.tensor_scalar(out=t_p, in0=negua2,
                                scalar1=ubT2[:, p:p + 1], scalar2=0.0,
                                op0=mybir.AluOpType.add,
                                op1=mybir.AluOpType.max)
            if p < N_WARMERS:
                nc.tensor.matmul(warm_ps, lhsT=t_p[0:C_IN, 0:C_OUT],
                                 rhs=t_p[0:C_IN, :], start=True, stop=True)
        for ph, (p0, p1) in enumerate(((0, 16), (16, 32))):
            observe_gather(ph)
            for p in range(p0, p1):
                rhs = g_sb[:, p, :]
                last = p == NPAIR - 1
                nc.tensor.matmul(acc[:, 0:C_OUT], lhsT=t_tiles[p][:, 0:128],
                                 rhs=rhs, start=False, stop=last)
                nc.tensor.matmul(acc[:, C_OUT:2 * C_OUT],
                                 lhsT=t_tiles[p][:, 128:256],
                                 rhs=rhs, start=False, stop=last)

        out_sb = work.tile([128, 2 * C_OUT], F32)
        nc.vector.tensor_copy(out_sb, acc)
        sponges += [nc.sync.nop(nofuse=True).ins for _ in range(2)]
        dst = bass.AP(tensor=outp.tensor, offset=outp.offset,
                      ap=[[C_OUT, 128], [128 * C_OUT, 2], [1, C_OUT]])
        nc.sync.dma_start(out=dst,
                          in_=out_sb.rearrange("p (h i) -> p h i", i=C_OUT))

    sponge_names = {sp.name for sp in sponges}
    for blk in nc.m.functions[0].blocks:
        insts = list(blk.instructions)
        for idx, ins in enumerate(insts):
            si = ins.sync_info
            if si is None or not si.on_wait or len(si.on_wait) <= 1:
                continue
            waits = list(si.on_wait)
            assert str(ins.engine).endswith("SP"), (
                f"multi-wait on non-SP instruction {ins.name} "
                f"{ins.engine} {ins.opcode}: "
                f"{[(w.ant_name, w.wait_value) for w in waits]}"
            )
            extras, keep = waits[:-1], waits[-1:]
            j = idx - 1
            while extras and j >= 0:
                prev = insts[j]
                j -= 1
                if prev.name not in sponge_names:
                    continue
                psi = prev.sync_info
                if psi is not None and psi.on_wait:
                    continue
                prev.sync_info = mybir.SyncInfo(on_wait=[extras.pop()],
                                                on_update=[])
            assert not extras, f"no sponge for {ins.name}"
            ins.sync_info = mybir.SyncInfo(on_wait=keep,
                                           on_update=list(si.on_update or []))
    return nc


def shard_inputs(features, geometry, W1, b1, W2, b2) -> list[dict]:
    import ml_dtypes
    bf16 = ml_dtypes.bfloat16
    f = np.ascontiguousarray(np.asarray(features, np.float32))
    g = np.ascontiguousarray(np.asarray(geometry, np.float32))
    W1 = np.ascontiguousarray(np.asarray(W1, np.float32))
    b1 = np.ascontiguousarray(np.asarray(b1, np.float32))
    W2 = np.ascontiguousarray(np.asarray(W2, np.float32))
    b2 = np.ascontiguousarray(np.asarray(b2, np.float32))

    m2p = W2.reshape(H, C_OUT, C_IN).transpose(2, 0, 1).reshape(C_IN, H * C_OUT)
    b2t = np.ascontiguousarray(b2.reshape(C_OUT, C_IN).T)

    maps = []
    for core in range(8):
        z, q = divmod(core, 4)
        sl = slice(q * BQ, (q + 1) * BQ)
        pkv = np.zeros((H, PKW), np.float32)
        pkv[0:C_IN, 0:256] = f[z].T
        if q == 0:
            pkv[0:C_IN, 256:288] = b2t
        pkv[0:H, 288] = b1
        mp = np.zeros((C_IN, MPW), bf16)
        mp[:, 0:64] = f[z, sl].T.astype(bf16)
        mp[0:3, 64:320] = g[z].T.astype(bf16)
        mp[0:3, 320:384] = g[z, sl].T.astype(bf16)
        mp[0:3, 384:448] = W1.astype(bf16)
        mp[:, 448:2496] = m2p.astype(bf16)
        maps.append({"pk": pkv, "M2p": mp})
    return maps


def unshard(parts: list[np.ndarray]) -> np.ndarray:
    out = np.empty((Z, N, C_OUT), np.float32)
    for z in range(Z):
        acc = parts[4 * z].astype(np.float32)
        for q in range(1, 4):
            acc = acc + parts[4 * z + q]
        out[z] = acc
    return out


def kernel(**inputs) -> np.ndarray:
    nc = build_nc(debug=False)
    in_maps = shard_inputs(**inputs)
    res = run_bass_kernel_spmd(nc, in_maps, list(range(8)))
    return unshard([r["outp"] for r in res.results])
